# revision 24
# baseline (speedup 1.0000x reference)
"""Trainium2 Bass kernel for BaseCPNN (vq_codebook):
  winners = argmin_h ||x_b - W_h||  == argmax_h (x_b . W_h)   [W rows unit-norm]
  output  = G.T[winners]            [row gather]

Data-parallel over batch across 8 NeuronCores: each core gets a 512-row shard
of x, the full codebook W^T and full G^T (replicated).

Matmul precision: the argmin needs fp32-grade scores (min top-2 gap ~3e-5 in
sq units). fp32 PE matmuls run at 4 cyc/row; instead we use an exact 3-pass
fp16 split: x = xh + xl, W = wh + wl (11-bit significand chunks, splits exact),
S = xh.wh + xh.wl + xl.wh accumulated in fp32 PSUM. Dropped xl.wl term and
residuals are ~2^-22 — two orders below the argmin gap. Each fp16 pass runs at
1 cyc/row, so 3 passes beat native fp32 by ~25% and halve the W DMA bytes.

Row argmax is computed streaming: per 512-wide h-chunk, vector-engine
max_with_indices produces the chunk top-1 (+index); a final 16-wide argmax
over chunk winners + one-hot select of the chunk-local index rebuilds the
global argmin with exact first-index tie semantics. Output rows are gathered
from G^T with indirect DMA using the winner indices.

Shapes (hardcoded): x [4096, 1024] f32, kohonen [8192, 1024] f32,
grossberg [1000, 8192] f32 -> (output [4096, 1000] f32, winners [4096] int32).
"""
import numpy as np
from contextlib import ExitStack

import concourse.bass as bass
import concourse.mybir as mybir
import concourse.tile as tile
from concourse import bacc
from concourse.bass_utils import run_bass_kernel_spmd


def _ensure_ntff_hook():
    """run_bass_kernel_spmd(trace=True) under axon hard-imports
    antenv.axon_hooks; some images lack it. Install a ctypes-based fallback
    (same mechanism trn_agent_boot uses) so tracing works when requested."""
    try:
        import antenv.axon_hooks  # noqa: F401
        return
    except ImportError:
        pass
    try:
        import sys, types
        import antenv
        from trn_agent_boot.trn_boot import _ntff_profile_via_ctypes
        state = {}

        def get_axon_ntff_profile_hook():
            if "h" not in state:
                state["h"] = _ntff_profile_via_ctypes("/opt/axon/libaxon_pjrt.so")
            return state["h"]

        def set_axon_ntff_profile_hook(h):
            state["h"] = h

        mod = types.ModuleType("antenv.axon_hooks")
        mod.get_axon_ntff_profile_hook = get_axon_ntff_profile_hook
        mod.set_axon_ntff_profile_hook = set_axon_ntff_profile_hook
        sys.modules["antenv.axon_hooks"] = mod
        antenv.axon_hooks = mod
    except Exception:
        pass


_ensure_ntff_hook()

B, D, H, O = 4096, 1024, 8192, 1000
NCORES = 8
BS = B // NCORES          # 512 batch rows per core
P = 128                   # partition width
NB = BS // P              # 4 batch tiles per core
ND = D // P               # 8 contraction tiles
HT = 512                  # h-chunk width (one psum bank)
NH = H // HT              # 16 h chunks

_cached = {}


def _build():
    if "nc" in _cached:
        return _cached["nc"]
    nc = bacc.Bacc("TRN2", target_bir_lowering=False, debug=False)
    f32 = mybir.dt.float32
    f16 = mybir.dt.float16
    u32 = mybir.dt.uint32
    i32 = mybir.dt.int32

    xh_d = nc.dram_tensor("xh", [D, BS], f16, kind="ExternalInput")
    xl_d = nc.dram_tensor("xl", [D, BS], f16, kind="ExternalInput")
    wh_d = nc.dram_tensor("wh", [D, H], f16, kind="ExternalInput")
    wl_d = nc.dram_tensor("wl", [D, H], f16, kind="ExternalInput")
    gT = nc.dram_tensor("gT", [H, O], f32, kind="ExternalInput")
    out = nc.dram_tensor("out", [BS, O], f32, kind="ExternalOutput")
    win = nc.dram_tensor("win", [BS, 1], i32, kind="ExternalOutput")

    with tile.TileContext(nc) as tc, ExitStack() as ctx:
        xpool = ctx.enter_context(tc.tile_pool(name="x", bufs=1))
        wpool = ctx.enter_context(tc.tile_pool(name="w", bufs=48))
        cpool = ctx.enter_context(tc.tile_pool(name="c", bufs=6))
        rpool = ctx.enter_context(tc.tile_pool(name="r", bufs=1))
        ipool = ctx.enter_context(tc.tile_pool(name="i", bufs=4))
        opool = ctx.enter_context(tc.tile_pool(name="o", bufs=8))
        ppool = ctx.enter_context(tc.tile_pool(name="p", bufs=8, space="PSUM"))

        # resident x chunk tiles [128(d), 512(b)] fp16. The d=0 tiles (the
        # first matmul group's gate) go on the sync/scalar queues ahead of the
        # W stream; the rest load on the gpsimd queue in parallel.
        xhs, xls = [], []
        for d in range(ND):
            t = xpool.tile([P, BS], f16, tag=f"xh{d}", name=f"xh{d}")
            (nc.sync if d == 0 else nc.gpsimd).dma_start(
                t[:], xh_d[d * P:(d + 1) * P, :])
            xhs.append(t)
            t = xpool.tile([P, BS], f16, tag=f"xl{d}", name=f"xl{d}")
            (nc.scalar if d == 0 else nc.gpsimd).dma_start(
                t[:], xl_d[d * P:(d + 1) * P, :])
            xls.append(t)

        # running best value / global index (f32) per b-tile
        best_v = [rpool.tile([P, 1], f32, tag=f"bv{bt}", name=f"bv{bt}")
                  for bt in range(NB)]
        best_i = [rpool.tile([P, 1], f32, tag=f"bi{bt}", name=f"bi{bt}")
                  for bt in range(NB)]

        for ht in range(NH):
            whs, wls = [], []
            for d in range(ND):
                w = wpool.tile([P, HT], f16, tag="w", name=f"wh{ht}_{d}")
                nc.sync.dma_start(
                    w[:], wh_d[d * P:(d + 1) * P, ht * HT:(ht + 1) * HT])
                whs.append(w)
                w = wpool.tile([P, HT], f16, tag="w", name=f"wl{ht}_{d}")
                nc.scalar.dma_start(
                    w[:], wl_d[d * P:(d + 1) * P, ht * HT:(ht + 1) * HT])
                wls.append(w)
            for bt in range(NB):
                bsl = slice(bt * P, (bt + 1) * P)
                psum = ppool.tile([P, HT], f32, tag="ps")
                k = 0
                for d in range(ND):
                    nc.tensor.matmul(psum[:], lhsT=xhs[d][:, bsl], rhs=whs[d][:],
                                     start=(k == 0), stop=False)
                    k += 1
                    nc.tensor.matmul(psum[:], lhsT=xhs[d][:, bsl], rhs=wls[d][:],
                                     start=False, stop=False)
                    k += 1
                    nc.tensor.matmul(psum[:], lhsT=xls[d][:, bsl], rhs=whs[d][:],
                                     start=False, stop=(d == ND - 1))
                    k += 1
                chunk = cpool.tile([P, HT], f32, tag="chunk")
                nc.vector.tensor_copy(chunk[:], psum[:])
                v8 = cpool.tile([P, 8], f32, tag="v8")
                i8 = cpool.tile([P, 8], u32, tag="i8")
                nc.vector.max_with_indices(v8[:], i8[:], chunk[:])
                # global index = local + ht*HT (exact in f32, values < 8192)
                cur_i = cpool.tile([P, 1], f32, tag="cur_i")
                nc.vector.tensor_copy(cur_i[:], i8[:, 0:1])
                if ht > 0:
                    nc.vector.tensor_scalar_add(cur_i[:], cur_i[:],
                                                float(ht * HT))
                    # strict > keeps the earlier (lower-h) winner on ties,
                    # matching argmin-first semantics
                    m = cpool.tile([P, 1], u32, tag="m")
                    nc.vector.tensor_tensor(out=m[:], in0=v8[:, 0:1],
                                            in1=best_v[bt][:],
                                            op=mybir.AluOpType.is_gt)
                    nc.vector.copy_predicated(best_v[bt][:], m[:], v8[:, 0:1])
                    nc.vector.copy_predicated(best_i[bt][:], m[:], cur_i[:])
                else:
                    nc.vector.tensor_copy(best_v[bt][:], v8[:, 0:1])
                    nc.vector.tensor_copy(best_i[bt][:], cur_i[:])

        for bt in range(NB):
            wini = ipool.tile([P, 1], i32, tag="wini")
            nc.vector.tensor_copy(wini[:], best_i[bt][:])
            nc.sync.dma_start(win[bt * P:(bt + 1) * P, :], wini[:])

            winu = ipool.tile([P, 1], u32, tag="winu")
            nc.vector.tensor_copy(winu[:], best_i[bt][:])
            g = opool.tile([P, O], f32, tag="g")
            nc.gpsimd.indirect_dma_start(
                out=g[:], out_offset=None, in_=gT[:],
                in_offset=bass.IndirectOffsetOnAxis(ap=winu[:, 0:1], axis=0))
            nc.sync.dma_start(out[bt * P:(bt + 1) * P, :], g[:])

    nc.compile()
    _cached["nc"] = nc
    return nc


def _split16(a):
    hi = a.astype(np.float16)
    lo = (a - hi.astype(np.float32)).astype(np.float16)
    return hi, lo


def _run(inputs, trace=False, **kw):
    nc = _build()
    x = np.asarray(inputs["x"], dtype=np.float32)
    W = np.asarray(inputs["kohonen_weights"], dtype=np.float32)
    G = np.asarray(inputs["grossberg_weights"], dtype=np.float32)
    WT = np.ascontiguousarray(W.T)                    # [D, H]
    wh, wl = _split16(WT)
    GT = np.ascontiguousarray(G.T)                    # [H, O]
    in_maps = []
    for c in range(NCORES):
        xT = np.ascontiguousarray(x[c * BS:(c + 1) * BS].T)   # [D, BS]
        xh, xl = _split16(xT)
        in_maps.append({"xh": xh, "xl": xl, "wh": wh, "wl": wl, "gT": GT})
    res = run_bass_kernel_spmd(nc, in_maps, list(range(NCORES)), trace=trace, **kw)
    output = np.concatenate([res.results[c]["out"] for c in range(NCORES)], axis=0)
    winners = np.concatenate(
        [res.results[c]["win"][:, 0] for c in range(NCORES)], axis=0).astype(np.int32)
    return (output, winners), res


def kernel(**inputs):
    (output, winners), _ = _run(inputs, trace=False)
    return (output, winners)


# revision 27
# speedup vs baseline: 1.0219x; 1.0219x over previous
"""Trainium2 Bass kernel for BaseCPNN (vq_codebook):
  winners = argmin_h ||x_b - W_h||  == argmax_h (x_b . W_h)   [W rows unit-norm]
  output  = G.T[winners]            [row gather]

Data-parallel over batch across 8 NeuronCores: each core gets a 512-row shard
of x, the full codebook W^T and full G^T (replicated).

Matmul precision: the argmin needs fp32-grade scores (min top-2 gap ~3e-5 in
sq units). fp32 PE matmuls run at 4 cyc/row; instead we use an exact 3-pass
fp16 split: x = xh + xl, W = wh + wl (11-bit significand chunks, splits exact),
S = xh.wh + xh.wl + xl.wh accumulated in fp32 PSUM. Dropped xl.wl term and
residuals are ~2^-22 — two orders below the argmin gap. Each fp16 pass runs at
1 cyc/row, so 3 passes beat native fp32 by ~25% and halve the W DMA bytes.

Row argmax is computed streaming: per 512-wide h-chunk, vector-engine
max_with_indices produces the chunk top-1 (+index), folded into a running
(best value, best index) pair with is_gt + copy_predicated — strict > keeps
the earlier chunk on ties, so exact argmin-first semantics are preserved.
Output rows are gathered from G^T with indirect DMA using the winner indices.
Input DMAs are spread across the sync/scalar/gpsimd descriptor queues so the
PE ramp is not paced by a single queue (~650ns/descriptor).

Shapes (hardcoded): x [4096, 1024] f32, kohonen [8192, 1024] f32,
grossberg [1000, 8192] f32 -> (output [4096, 1000] f32, winners [4096] int32).
"""
import numpy as np
from contextlib import ExitStack

import concourse.bass as bass
import concourse.mybir as mybir
import concourse.tile as tile
from concourse import bacc
from concourse.bass_utils import run_bass_kernel_spmd


def _ensure_ntff_hook():
    """run_bass_kernel_spmd(trace=True) under axon hard-imports
    antenv.axon_hooks; some images lack it. Install a ctypes-based fallback
    (same mechanism trn_agent_boot uses) so tracing works when requested."""
    try:
        import antenv.axon_hooks  # noqa: F401
        return
    except ImportError:
        pass
    try:
        import sys, types
        import antenv
        from trn_agent_boot.trn_boot import _ntff_profile_via_ctypes
        state = {}

        def get_axon_ntff_profile_hook():
            if "h" not in state:
                state["h"] = _ntff_profile_via_ctypes("/opt/axon/libaxon_pjrt.so")
            return state["h"]

        def set_axon_ntff_profile_hook(h):
            state["h"] = h

        mod = types.ModuleType("antenv.axon_hooks")
        mod.get_axon_ntff_profile_hook = get_axon_ntff_profile_hook
        mod.set_axon_ntff_profile_hook = set_axon_ntff_profile_hook
        sys.modules["antenv.axon_hooks"] = mod
        antenv.axon_hooks = mod
    except Exception:
        pass


_ensure_ntff_hook()

B, D, H, O = 4096, 1024, 8192, 1000
NCORES = 8
BS = B // NCORES          # 512 batch rows per core
P = 128                   # partition width
NB = BS // P              # 4 batch tiles per core
ND = D // P               # 8 contraction tiles
HT = 512                  # h-chunk width (one psum bank)
NH = H // HT              # 16 h chunks

_cached = {}


def _build():
    if "nc" in _cached:
        return _cached["nc"]
    nc = bacc.Bacc("TRN2", target_bir_lowering=False, debug=False)
    f32 = mybir.dt.float32
    f16 = mybir.dt.float16
    u32 = mybir.dt.uint32
    i32 = mybir.dt.int32

    xh_d = nc.dram_tensor("xh", [D, BS], f16, kind="ExternalInput")
    xl_d = nc.dram_tensor("xl", [D, BS], f16, kind="ExternalInput")
    wh_d = nc.dram_tensor("wh", [D, H], f16, kind="ExternalInput")
    wl_d = nc.dram_tensor("wl", [D, H], f16, kind="ExternalInput")
    gT = nc.dram_tensor("gT", [H, O], f32, kind="ExternalInput")
    out = nc.dram_tensor("out", [BS, O], f32, kind="ExternalOutput")
    win = nc.dram_tensor("win", [BS, 1], i32, kind="ExternalOutput")

    with tile.TileContext(nc) as tc, ExitStack() as ctx:
        xpool = ctx.enter_context(tc.tile_pool(name="x", bufs=1))
        wpool = ctx.enter_context(tc.tile_pool(name="w", bufs=48))
        cpool = ctx.enter_context(tc.tile_pool(name="c", bufs=6))
        rpool = ctx.enter_context(tc.tile_pool(name="r", bufs=1))
        ipool = ctx.enter_context(tc.tile_pool(name="i", bufs=4))
        opool = ctx.enter_context(tc.tile_pool(name="o", bufs=8))
        ppool = ctx.enter_context(tc.tile_pool(name="p", bufs=8, space="PSUM"))

        # resident x chunk tiles [128(d), 512(b)] fp16. The d=0 tiles (the
        # first matmul group's gate) go on the sync/scalar queues ahead of the
        # W stream; the rest load on the gpsimd queue in parallel.
        xhs, xls = [], []
        for d in range(ND):
            t = xpool.tile([P, BS], f16, tag=f"xh{d}", name=f"xh{d}")
            (nc.sync if d == 0 else nc.gpsimd).dma_start(
                t[:], xh_d[d * P:(d + 1) * P, :])
            xhs.append(t)
            t = xpool.tile([P, BS], f16, tag=f"xl{d}", name=f"xl{d}")
            (nc.scalar if d == 0 else nc.gpsimd).dma_start(
                t[:], xl_d[d * P:(d + 1) * P, :])
            xls.append(t)

        # running best value / global index (f32) per b-tile
        best_v = [rpool.tile([P, 1], f32, tag=f"bv{bt}", name=f"bv{bt}")
                  for bt in range(NB)]
        best_i = [rpool.tile([P, 1], f32, tag=f"bi{bt}", name=f"bi{bt}")
                  for bt in range(NB)]

        for ht in range(NH):
            whs, wls = [], []
            for d in range(ND):
                w = wpool.tile([P, HT], f16, tag="w", name=f"wh{ht}_{d}")
                nc.sync.dma_start(
                    w[:], wh_d[d * P:(d + 1) * P, ht * HT:(ht + 1) * HT])
                whs.append(w)
                w = wpool.tile([P, HT], f16, tag="w", name=f"wl{ht}_{d}")
                nc.scalar.dma_start(
                    w[:], wl_d[d * P:(d + 1) * P, ht * HT:(ht + 1) * HT])
                wls.append(w)
            for bt in range(NB):
                bsl = slice(bt * P, (bt + 1) * P)
                psum = ppool.tile([P, HT], f32, tag="ps")
                k = 0
                for d in range(ND):
                    nc.tensor.matmul(psum[:], lhsT=xhs[d][:, bsl], rhs=whs[d][:],
                                     start=(k == 0), stop=False)
                    k += 1
                    nc.tensor.matmul(psum[:], lhsT=xhs[d][:, bsl], rhs=wls[d][:],
                                     start=False, stop=False)
                    k += 1
                    nc.tensor.matmul(psum[:], lhsT=xls[d][:, bsl], rhs=whs[d][:],
                                     start=False, stop=(d == ND - 1))
                    k += 1
                chunk = cpool.tile([P, HT], f32, tag="chunk")
                nc.vector.tensor_copy(chunk[:], psum[:])
                v8 = cpool.tile([P, 8], f32, tag="v8")
                i8 = cpool.tile([P, 8], u32, tag="i8")
                nc.vector.max_with_indices(v8[:], i8[:], chunk[:])
                # global index = local + ht*HT (exact in f32, values < 8192)
                cur_i = cpool.tile([P, 1], f32, tag="cur_i")
                nc.vector.tensor_copy(cur_i[:], i8[:, 0:1])
                if ht > 0:
                    nc.vector.tensor_scalar_add(cur_i[:], cur_i[:],
                                                float(ht * HT))
                    # strict > keeps the earlier (lower-h) winner on ties,
                    # matching argmin-first semantics
                    m = cpool.tile([P, 1], u32, tag="m")
                    nc.vector.tensor_tensor(out=m[:], in0=v8[:, 0:1],
                                            in1=best_v[bt][:],
                                            op=mybir.AluOpType.is_gt)
                    nc.vector.copy_predicated(best_v[bt][:], m[:], v8[:, 0:1])
                    nc.vector.copy_predicated(best_i[bt][:], m[:], cur_i[:])
                else:
                    nc.vector.tensor_copy(best_v[bt][:], v8[:, 0:1])
                    nc.vector.tensor_copy(best_i[bt][:], cur_i[:])

        for bt in range(NB):
            wini = ipool.tile([P, 1], i32, tag="wini")
            nc.vector.tensor_copy(wini[:], best_i[bt][:])
            nc.sync.dma_start(win[bt * P:(bt + 1) * P, :], wini[:])

            winu = ipool.tile([P, 1], u32, tag="winu")
            nc.vector.tensor_copy(winu[:], best_i[bt][:])
            g = opool.tile([P, O], f32, tag="g")
            nc.gpsimd.indirect_dma_start(
                out=g[:], out_offset=None, in_=gT[:],
                in_offset=bass.IndirectOffsetOnAxis(ap=winu[:, 0:1], axis=0))
            nc.sync.dma_start(out[bt * P:(bt + 1) * P, :], g[:])

    nc.compile()
    _cached["nc"] = nc
    return nc


def _split16(a):
    hi = a.astype(np.float16)
    lo = (a - hi.astype(np.float32)).astype(np.float16)
    return hi, lo


def _run(inputs, trace=False, **kw):
    nc = _build()
    x = np.asarray(inputs["x"], dtype=np.float32)
    W = np.asarray(inputs["kohonen_weights"], dtype=np.float32)
    G = np.asarray(inputs["grossberg_weights"], dtype=np.float32)
    WT = np.ascontiguousarray(W.T)                    # [D, H]
    wh, wl = _split16(WT)
    GT = np.ascontiguousarray(G.T)                    # [H, O]
    in_maps = []
    for c in range(NCORES):
        xT = np.ascontiguousarray(x[c * BS:(c + 1) * BS].T)   # [D, BS]
        xh, xl = _split16(xT)
        in_maps.append({"xh": xh, "xl": xl, "wh": wh, "wl": wl, "gT": GT})
    res = run_bass_kernel_spmd(nc, in_maps, list(range(NCORES)), trace=trace, **kw)
    output = np.concatenate([res.results[c]["out"] for c in range(NCORES)], axis=0)
    winners = np.concatenate(
        [res.results[c]["win"][:, 0] for c in range(NCORES)], axis=0).astype(np.int32)
    return (output, winners), res


def kernel(**inputs):
    (output, winners), _ = _run(inputs, trace=False)
    return (output, winners)
